# revision 1
# baseline (speedup 1.0000x reference)
"""2D DCT-II (4096x4096) on 8 Trainium2 NeuronCores (axon/PJRT SPMD).

Math: the reference computes C = G_M @ y @ G_N^T, y = x[pm][:, pn] (Makhoul
even-odd reorder), G built from the DFT kernel and the expk inputs:
  G_M[u,i] = 0.5*(eMr[u]*cos(2pi*u*i/M) + eMi[u]*sin(2pi*u*i/M))
  G_N[v,j] = 2.0*(eNr[v]*cos(2pi*v*j/N) + eNi[v]*sin(2pi*v*j/N))
Permutations fold into the tables (A[:, pm[i]] = G[:, i]), so on device:
  C = A_M @ x @ A_N^T        -- two dense 4096^3 matmuls.

Distribution (8 cores):
  phase 1: core k computes W_k = x[rows_k, :] @ A_N^T  (rows_k = 512k..+512),
           written in 8 column blocks [8, 512, 512] for the exchange.
  AllToAll: block (k -> j): W_k[:, cols_j]; after, core k holds
           W[:, cols_k] as [8, 512, 512] (m-th block = rows 512m).
  phase 2: core k computes C[:, cols_k] = A_M @ W[:, cols_k].
Host: builds A tables from expk (cached), slices x^T, concatenates shards.
Matmuls run as float32r (TF32-like, ~1e-4 rel err) via AP bitcast.
"""
import numpy as np

_NCORES = 8
_SZ = 4096
_RPC = _SZ // _NCORES  # 512 rows/cols per core
_KT = _SZ // 128       # 32 contraction tiles

_state = {}


# --------------------------------------------------------------------------
# Bass kernel
# --------------------------------------------------------------------------
def _build_bass(sz=_SZ):
    import concourse.bacc as bacc
    import concourse.mybir as mybir
    from concourse.tile import TileContext

    fp32 = mybir.dt.float32
    fp32r = mybir.dt.float32r
    _SZ = sz
    _RPC = _SZ // _NCORES
    _KT = _SZ // 128
    nc = bacc.Bacc("TRN2", target_bir_lowering=False, debug=False,
                   num_devices=_NCORES)
    xT = nc.declare_dram_parameter("xT", [_SZ, _RPC], fp32r, isOutput=False)
    annT = nc.declare_dram_parameter("annT", [_SZ, _SZ], fp32r, isOutput=False)
    amT = nc.declare_dram_parameter("amT", [_SZ, _SZ], fp32r, isOutput=False)
    cout = nc.declare_dram_parameter("cout", [_SZ, _RPC], fp32, isOutput=True)

    w_send = nc.dram_tensor("w_send", [_NCORES, _RPC, _RPC], fp32r)
    w_recv = nc.dram_tensor("w_recv", [_NCORES, _RPC, _RPC], fp32r)

    with TileContext(nc) as tc:
        # ---------- phase 1: W_k = x_k @ A_N^T ----------
        # xT resident in SBUF as [128, (kt, r)] : xT[kt*128+p, r]
        # annT streamed in 256-wide v panels [128, (kt, v)]
        with (
            tc.tile_pool(name="xw", bufs=1) as xw_pool,
            tc.tile_pool(name="an", bufs=3) as an_pool,
            tc.tile_pool(name="ps1", bufs=8, space="PSUM") as ps1_pool,
            tc.tile_pool(name="ev1", bufs=4) as ev1_pool,
        ):
            xw = xw_pool.tile([128, _KT * _RPC], fp32r)  # 8 MB
            nc.sync.dma_start(
                out=xw[:].rearrange("p (kt r) -> p kt r", kt=_KT),
                in_=xT[:].rearrange("(kt p) r -> p kt r", p=128))
            VP = min(256, _RPC)
            for vp in range(_SZ // VP):
                an = an_pool.tile([128, _KT * VP], fp32r, tag="an")  # 4 MB
                nc.sync.dma_start(
                    out=an[:].rearrange("p (kt v) -> p kt v", kt=_KT),
                    in_=annT[:, vp * VP:(vp + 1) * VP]
                    .rearrange("(kt p) v -> p kt v", p=128))
                for rt in range(_RPC // 128):
                    ps = ps1_pool.tile([128, VP], fp32, tag="ps")
                    for k in range(_KT):
                        nc.tensor.matmul(
                            ps[:],
                            xw[:, k * _RPC + rt * 128:
                                  k * _RPC + rt * 128 + 128],
                            an[:, k * VP:(k + 1) * VP],
                            start=(k == 0), stop=(k == _KT - 1))
                    ev = ev1_pool.tile([128, VP], fp32r, tag="ev")
                    nc.vector.tensor_copy(ev[:], ps[:])
                    # rows rt*128..+128 of W_k, cols vp*VP..+VP ->
                    # w_send[block j, r, c] with j = col//512
                    j = (vp * VP) // _RPC
                    c0 = (vp * VP) % _RPC
                    nc.sync.dma_start(
                        out=w_send[j, rt * 128:(rt + 1) * 128, c0:c0 + VP],
                        in_=ev[:])

        # ---------- exchange ----------
        nc.gpsimd.collective_compute(
            "AllToAll",
            mybir.AluOpType.bypass,
            ins=[w_send[:]],
            outs=[w_recv[:]],
            replica_groups=[list(range(_NCORES))],
        )

        # ---------- phase 2: C[:, cols_k] = A_M @ W[:, cols_k] ----------
        # w_recv resident [128, (kt, v)]: global row i = kt*128 + p
        #   w_recv[m, i2, v], m = kt//4, i2 = (kt%4)*128 + p
        # amT streamed per u-tile [128, (kt, u)]
        with (
            tc.tile_pool(name="wr", bufs=1) as wr_pool,
            tc.tile_pool(name="am", bufs=3) as am_pool,
            tc.tile_pool(name="ps2", bufs=8, space="PSUM") as ps2_pool,
            tc.tile_pool(name="ev2", bufs=4) as ev2_pool,
        ):
            wr = wr_pool.tile([128, _KT * _RPC], fp32r)  # 8 MB
            nc.sync.dma_start(
                out=wr[:].rearrange("p (m s v) -> p m s v", m=_NCORES, s=4),
                in_=w_recv[:].rearrange("m (s p) v -> p m s v", p=128))
            for ut in range(_SZ // 128):
                am = am_pool.tile([128, _KT * 128], fp32r, tag="am")  # 2 MB
                nc.sync.dma_start(
                    out=am[:].rearrange("p (kt u) -> p kt u", kt=_KT),
                    in_=amT[:, ut * 128:(ut + 1) * 128]
                    .rearrange("(kt p) u -> p kt u", p=128))
                VP2 = min(256, _RPC)
                for half in range(_RPC // VP2):
                    ps = ps2_pool.tile([128, VP2], fp32, tag="ps2")
                    for k in range(_KT):
                        nc.tensor.matmul(
                            ps[:],
                            am[:, k * 128:(k + 1) * 128],
                            wr[:, k * _RPC + half * VP2:
                                  k * _RPC + (half + 1) * VP2],
                            start=(k == 0), stop=(k == _KT - 1))
                    ev = ev2_pool.tile([128, VP2], fp32, tag="ev2")
                    nc.vector.tensor_copy(ev[:], ps[:])
                    nc.sync.dma_start(
                        out=cout[ut * 128:(ut + 1) * 128,
                                 half * VP2:(half + 1) * VP2],
                        in_=ev[:])

    nc.compile()
    return nc


# --------------------------------------------------------------------------
# PJRT SPMD runner (compile once, run many)
# --------------------------------------------------------------------------
def _build_runner(nc, n_cores):
    import jax
    from jax.sharding import Mesh, PartitionSpec
    from jax.experimental.shard_map import shard_map
    import concourse.mybir as mybir
    from concourse import bass2jax
    from concourse.bass2jax import _bass_exec_p, partition_id_tensor

    bass2jax.install_neuronx_cc_hook()
    partition_name = (nc.partition_id_tensor.name
                      if nc.partition_id_tensor else None)

    in_names, out_names, out_avals, zero_outs = [], [], [], []
    for alloc in nc.m.functions[0].allocations:
        if not isinstance(alloc, mybir.MemoryLocationSet):
            continue
        name = alloc.memorylocations[0].name
        if alloc.kind == "ExternalInput":
            if name != partition_name:
                in_names.append(name)
        elif alloc.kind == "ExternalOutput":
            shape = tuple(alloc.tensor_shape)
            dtype = mybir.dt.np(alloc.dtype)
            out_names.append(name)
            out_avals.append(jax.core.ShapedArray(shape, dtype))
            zero_outs.append(np.zeros(shape, dtype))
    n_params = len(in_names)
    n_outs = len(out_avals)
    in_names_all = list(in_names) + out_names
    if partition_name is not None:
        in_names_all = in_names_all + [partition_name]
    donate = tuple(range(n_params, n_params + n_outs))

    def _body(*args):
        operands = list(args)
        if partition_name is not None:
            operands.append(partition_id_tensor())
        outs = _bass_exec_p.bind(
            *operands,
            out_avals=tuple(out_avals),
            in_names=tuple(in_names_all),
            out_names=tuple(out_names),
            lowering_input_output_aliases=(),
            sim_require_finite=True,
            sim_require_nnan=True,
            nc=nc,
        )
        return tuple(outs)

    devices = jax.devices()[:n_cores]
    mesh = Mesh(np.asarray(devices), ("core",))
    sharded = jax.jit(
        shard_map(_body, mesh=mesh,
                  in_specs=(PartitionSpec("core"),) * (n_params + n_outs),
                  out_specs=(PartitionSpec("core"),) * n_outs,
                  check_rep=False),
        donate_argnums=donate, keep_unused=True)

    from jax.sharding import NamedSharding
    shard = NamedSharding(mesh, PartitionSpec("core"))
    _dev_cache = {}

    import jax.numpy as jnp
    _zero_shapes = [(n_cores * z.shape[0], *z.shape[1:]) for z in zero_outs]
    _zero_dtypes = [z.dtype for z in zero_outs]

    _make_zeros = jax.jit(
        lambda: tuple(jnp.zeros(s, d)
                      for s, d in zip(_zero_shapes, _zero_dtypes)),
        out_shardings=(shard,) * len(_zero_shapes))

    def run(in_maps, cache_names=(), fetch=True):
        concat_in = []
        for i, name in enumerate(in_names):
            if name in cache_names and name in _dev_cache:
                concat_in.append(_dev_cache[name])
                continue
            arr = np.concatenate(
                [np.asarray(in_maps[c][name]) for c in range(n_cores)], axis=0)
            arr = jax.device_put(arr, shard)
            if name in cache_names:
                jax.block_until_ready(arr)
                _dev_cache[name] = arr
            concat_in.append(arr)
        concat_zeros = _make_zeros()
        raw = sharded(*concat_in, *concat_zeros)
        if not fetch:
            import jax as _jax
            _jax.block_until_ready(raw)
            return raw
        out_arrs = [np.asarray(o) for o in raw]
        return [
            {name: out_arrs[i].reshape(n_cores, *out_avals[i].shape)[c]
             for i, name in enumerate(out_names)}
            for c in range(n_cores)]

    run.dev_cache = _dev_cache
    return run


# --------------------------------------------------------------------------
# host-side tables
# --------------------------------------------------------------------------
def _tables(expkM, expkN):
    key = (expkM.tobytes(), expkN.tobytes())
    cached = _state.get("tables")
    if cached is not None and cached[0] == key:
        return cached[1], cached[2]
    run = _state.get("run")
    if run is not None:
        run.dev_cache.clear()
    n = _SZ
    i = np.arange(n)
    pm = np.where(i < (n + 1) // 2, 2 * i, 2 * (n - i) - 1)
    pinv = np.empty(n, dtype=np.int64)
    pinv[pm] = i
    # Cp[j, v] = cos(2pi * pinv[j] * v / n); note cos/sin tables are symmetric
    ang = (2.0 * np.pi / n) * np.outer(pinv.astype(np.float64),
                                       i.astype(np.float64))
    Cp = np.cos(ang)
    Sp = np.sin(ang)
    eMr = expkM[:, 0].astype(np.float64)
    eMi = expkM[:, 1].astype(np.float64)
    eNr = expkN[:, 0].astype(np.float64)
    eNi = expkN[:, 1].astype(np.float64)
    annT = np.ascontiguousarray(
        (2.0 * (Cp * eNr[None, :] + Sp * eNi[None, :])).astype(np.float32))
    amT = np.ascontiguousarray(
        (0.5 * (Cp * eMr[None, :] + Sp * eMi[None, :])).astype(np.float32))
    _state["tables"] = (key, annT, amT)
    return annT, amT


def kernel(x, expkM, expkN, M, N):
    x = np.asarray(x, dtype=np.float32)
    expkM = np.asarray(expkM, dtype=np.float32)
    expkN = np.asarray(expkN, dtype=np.float32)
    assert x.shape == (_SZ, _SZ)

    annT, amT = _tables(expkM, expkN)
    if "run" not in _state:
        _state["run"] = _build_runner(_build_bass(), _NCORES)
    run = _state["run"]

    xT = np.ascontiguousarray(x.T)
    in_maps = [
        {"xT": np.ascontiguousarray(xT[:, k * _RPC:(k + 1) * _RPC]),
         "annT": annT, "amT": amT}
        for k in range(_NCORES)]
    outs = run(in_maps, cache_names=("annT", "amT"))
    C = np.concatenate([outs[k]["cout"] for k in range(_NCORES)], axis=1)
    return np.ascontiguousarray(C, dtype=np.float32)



# revision 4
# speedup vs baseline: 767.6141x; 767.6141x over previous
"""2D DCT-II (4096x4096) on 8 Trainium2 NeuronCores (axon/PJRT SPMD).

Math: the reference computes C = A_M @ x @ A_N^T, where y = x[pm][:, pn]
(Makhoul even-odd reorder) is folded into the tables (A[:, pm[i]] = G[:, i]):
  G_M[u,i] = 0.5*(eMr[u]*cos(2pi*u*i/M) + eMi[u]*sin(2pi*u*i/M))
  G_N[v,j] = 2.0*(eNr[v]*cos(2pi*v*j/N) + eNi[v]*sin(2pi*v*j/N))
On device (per core k, rows_k = 512k..512k+512):
  AllGather x (each core gets the full 4096x4096 x in HBM), then locally
  T1^T = (A_M[rows_k,:] @ x)^T  via stationary=x-tiles, moving=amTs (SBUF),
  C[rows_k,:] = T1 @ A_N^T      via stationary=T1^T-tiles, moving=annT.
Input x is row-sharded (zero host copies), output C is row-sharded (the
fetched stacked array IS C). Tables are cached on device across calls.
Matmuls run as float32r (TF32-like, ~2e-4 rel err) via AP bitcast.
"""
import numpy as np

_NCORES = 8
_SZ = 4096
_RPC = _SZ // _NCORES  # 512 rows per core
_KT = _SZ // 128       # 32 contraction tiles

_state = {}


# --------------------------------------------------------------------------
# Bass kernel
# --------------------------------------------------------------------------
def _build_bass():
    import concourse.bacc as bacc
    import concourse.mybir as mybir
    from concourse.tile import TileContext

    fp32 = mybir.dt.float32
    fp32r = mybir.dt.float32r
    VP = 512               # moving-panel width (max moving free dim)
    NVP = _SZ // VP        # 8
    UT = _RPC // 128       # 4 u-tiles per core
    CB = 4                 # concurrent PSUM accumulators in phase 1

    nc = bacc.Bacc("TRN2", target_bir_lowering=False, debug=False,
                   num_devices=_NCORES)
    xk = nc.declare_dram_parameter("xk", [_RPC, _SZ], fp32r, isOutput=False)
    annT = nc.declare_dram_parameter("annT", [_SZ, _SZ], fp32r, isOutput=False)
    amTs = nc.declare_dram_parameter("amTs", [_SZ, _RPC], fp32r, isOutput=False)
    cout = nc.declare_dram_parameter("cout", [_RPC, _SZ], fp32, isOutput=True)

    xg_send = nc.dram_tensor("xg_send", [_RPC, _SZ], fp32r)
    xg_full = nc.dram_tensor("xg_full", [_SZ, _SZ], fp32r,
                             addr_space="Shared")

    with TileContext(nc) as tc:
        # bounce x_k into internal DRAM (collectives can't touch kernel I/O)
        with tc.tile_pool(name="xb", bufs=2) as xb_pool:
            for rt in range(UT):
                xb = xb_pool.tile([128, _SZ], fp32r, tag="xb")
                nc.sync.dma_start(out=xb[:], in_=xk[rt * 128:(rt + 1) * 128, :])
                nc.sync.dma_start(out=xg_send[rt * 128:(rt + 1) * 128, :],
                                  in_=xb[:])

        nc.gpsimd.collective_compute(
            "AllGather",
            mybir.AluOpType.bypass,
            ins=[xg_send[:]],
            outs=[xg_full[:]],
            replica_groups=[list(range(_NCORES))],
        )

        with (
            tc.tile_pool(name="ams", bufs=1) as ams_pool,
            tc.tile_pool(name="t1", bufs=1) as t1_pool,
            tc.tile_pool(name="xp", bufs=3) as xp_pool,
            tc.tile_pool(name="anp", bufs=3) as anp_pool,
            tc.tile_pool(name="ps", bufs=8, space="PSUM") as ps_pool,
            tc.tile_pool(name="ev", bufs=4) as ev_pool,
        ):
            # amTs resident in SBUF: ams[i0, it, u] = amTs[it*128+i0, u]
            ams = ams_pool.tile([128, _KT * _RPC], fp32r)  # 8 MB
            nc.sync.dma_start(
                out=ams[:].rearrange("p (it u) -> p it u", it=_KT),
                in_=amTs[:].rearrange("(it p) u -> p it u", p=128))
            # T1^T resident: t1[c0, ct, u] = T1[u, ct*128+c0]
            t1 = t1_pool.tile([128, _KT * _RPC], fp32r)    # 8 MB

            # phase 1: T1^T[c, u] = sum_i x[i, c] * amTs[i, u]
            for cb in range(_SZ // (CB * 128)):            # 8 column blocks
                pss = [ps_pool.tile([128, _RPC], fp32, tag="ps",
                                    name=f"ps_{cb}_{ci}")
                       for ci in range(CB)]
                for it in range(_KT):
                    xp = xp_pool.tile([128, CB * 128], fp32r, tag="xp")
                    nc.sync.dma_start(
                        out=xp[:],
                        in_=xg_full[it * 128:(it + 1) * 128,
                                    cb * CB * 128:(cb + 1) * CB * 128])
                    for ci in range(CB):
                        nc.tensor.matmul(
                            pss[ci][:],
                            xp[:, ci * 128:(ci + 1) * 128],
                            ams[:, it * _RPC:(it + 1) * _RPC],
                            start=(it == 0), stop=(it == _KT - 1))
                for ci in range(CB):
                    ct = cb * CB + ci
                    nc.vector.tensor_copy(
                        t1[:, ct * _RPC:(ct + 1) * _RPC], pss[ci][:])

            # phase 2: C[u, v] = sum_c T1^T[c, u] * annT[c, v]
            for vp in range(NVP):                          # 8 v-panels
                qss = [ps_pool.tile([128, VP], fp32, tag="ps",
                                    name=f"qs_{vp}_{ut}")
                       for ut in range(UT)]
                for ct in range(_KT):
                    anp = anp_pool.tile([128, VP], fp32r, tag="anp")
                    nc.sync.dma_start(
                        out=anp[:],
                        in_=annT[ct * 128:(ct + 1) * 128,
                                 vp * VP:(vp + 1) * VP])
                    for ut in range(UT):
                        nc.tensor.matmul(
                            qss[ut][:],
                            t1[:, ct * _RPC + ut * 128:
                                  ct * _RPC + (ut + 1) * 128],
                            anp[:],
                            start=(ct == 0), stop=(ct == _KT - 1))
                for ut in range(UT):
                    ev = ev_pool.tile([128, VP], fp32, tag="ev")
                    nc.vector.tensor_copy(ev[:], qss[ut][:])
                    nc.sync.dma_start(
                        out=cout[ut * 128:(ut + 1) * 128,
                                 vp * VP:(vp + 1) * VP],
                        in_=ev[:])

    nc.compile()
    return nc


# --------------------------------------------------------------------------
# PJRT SPMD runner (compile once, run many)
# --------------------------------------------------------------------------
def _build_runner(nc, n_cores, replicated_names=()):
    import jax
    from jax.sharding import Mesh, NamedSharding, PartitionSpec
    from jax.experimental.shard_map import shard_map
    import concourse.mybir as mybir
    from concourse import bass2jax
    from concourse.bass2jax import _bass_exec_p, partition_id_tensor

    bass2jax.install_neuronx_cc_hook()
    partition_name = (nc.partition_id_tensor.name
                      if nc.partition_id_tensor else None)

    in_names, out_names, out_avals, zero_outs = [], [], [], []
    for alloc in nc.m.functions[0].allocations:
        if not isinstance(alloc, mybir.MemoryLocationSet):
            continue
        name = alloc.memorylocations[0].name
        if alloc.kind == "ExternalInput":
            if name != partition_name:
                in_names.append(name)
        elif alloc.kind == "ExternalOutput":
            shape = tuple(alloc.tensor_shape)
            dtype = mybir.dt.np(alloc.dtype)
            out_names.append(name)
            out_avals.append(jax.core.ShapedArray(shape, dtype))
            zero_outs.append(np.zeros(shape, dtype))
    n_params = len(in_names)
    n_outs = len(out_avals)
    in_names_all = list(in_names) + out_names
    if partition_name is not None:
        in_names_all = in_names_all + [partition_name]
    donate = tuple(range(n_params, n_params + n_outs))

    def _body(*args):
        operands = list(args)
        if partition_name is not None:
            operands.append(partition_id_tensor())
        outs = _bass_exec_p.bind(
            *operands,
            out_avals=tuple(out_avals),
            in_names=tuple(in_names_all),
            out_names=tuple(out_names),
            lowering_input_output_aliases=(),
            sim_require_finite=True,
            sim_require_nnan=True,
            nc=nc,
        )
        return tuple(outs)

    devices = jax.devices()[:n_cores]
    mesh = Mesh(np.asarray(devices), ("core",))
    spec_row = PartitionSpec("core")
    spec_rep = PartitionSpec()
    in_specs = tuple(spec_rep if nm in replicated_names else spec_row
                     for nm in in_names)
    sharded = jax.jit(
        shard_map(_body, mesh=mesh,
                  in_specs=in_specs + (spec_row,) * n_outs,
                  out_specs=(spec_row,) * n_outs,
                  check_rep=False),
        donate_argnums=donate, keep_unused=True)

    shard = NamedSharding(mesh, spec_row)
    shard_rep = NamedSharding(mesh, spec_rep)
    _dev_cache = {}

    import jax.numpy as jnp
    _zero_shapes = [(n_cores * z.shape[0], *z.shape[1:]) for z in zero_outs]
    _zero_dtypes = [z.dtype for z in zero_outs]

    _make_zeros = jax.jit(
        lambda: tuple(jnp.zeros(s, d)
                      for s, d in zip(_zero_shapes, _zero_dtypes)),
        out_shardings=(shard,) * len(_zero_shapes))

    def run(stacked_in, cache_names=(), fetch=True):
        """stacked_in: dict name -> FULL stacked np array (replicated names
        get the per-core array as-is)."""
        concat_in = []
        for name in in_names:
            if name in cache_names and name in _dev_cache:
                concat_in.append(_dev_cache[name])
                continue
            sh = shard_rep if name in replicated_names else shard
            arr = jax.device_put(stacked_in[name], sh)
            if name in cache_names:
                jax.block_until_ready(arr)
                _dev_cache[name] = arr
            concat_in.append(arr)
        concat_zeros = _make_zeros()
        raw = sharded(*concat_in, *concat_zeros)
        if not fetch:
            jax.block_until_ready(raw)
            return raw
        return [np.asarray(o) for o in raw]

    run.dev_cache = _dev_cache
    run.out_names = out_names
    return run


# --------------------------------------------------------------------------
# host-side tables
# --------------------------------------------------------------------------
def _tables(expkM, expkN):
    key = (expkM.tobytes(), expkN.tobytes())
    cached = _state.get("tables")
    if cached is not None and cached[0] == key:
        return cached[1], cached[2]
    run = _state.get("run")
    if run is not None:
        run.dev_cache.clear()
    n = _SZ
    i = np.arange(n)
    pm = np.where(i < (n + 1) // 2, 2 * i, 2 * (n - i) - 1)
    pinv = np.empty(n, dtype=np.int64)
    pinv[pm] = i
    # Cp[j, v] = cos(2pi * pinv[j] * v / n)
    ang = (2.0 * np.pi / n) * np.outer(pinv.astype(np.float64),
                                       i.astype(np.float64))
    Cp = np.cos(ang)
    Sp = np.sin(ang)
    eMr = expkM[:, 0].astype(np.float64)
    eMi = expkM[:, 1].astype(np.float64)
    eNr = expkN[:, 0].astype(np.float64)
    eNi = expkN[:, 1].astype(np.float64)
    annT = np.ascontiguousarray(
        (2.0 * (Cp * eNr[None, :] + Sp * eNi[None, :])).astype(np.float32))
    amT = np.ascontiguousarray(
        (0.5 * (Cp * eMr[None, :] + Sp * eMi[None, :])).astype(np.float32))
    # amTs stacked: core k gets amT[:, k*512:(k+1)*512]
    amTs = np.ascontiguousarray(
        amT.reshape(n, _NCORES, _RPC).transpose(1, 0, 2)
    ).reshape(_NCORES * n, _RPC)
    _state["tables"] = (key, annT, amTs)
    return annT, amTs


def kernel(x, expkM, expkN, M, N):
    x = np.ascontiguousarray(np.asarray(x, dtype=np.float32))
    expkM = np.asarray(expkM, dtype=np.float32)
    expkN = np.asarray(expkN, dtype=np.float32)
    assert x.shape == (_SZ, _SZ)

    annT, amTs = _tables(expkM, expkN)
    if "run" not in _state:
        _state["run"] = _build_runner(_build_bass(), _NCORES,
                                      replicated_names=("annT",))
    run = _state["run"]

    outs = run({"xk": x, "annT": annT, "amTs": amTs},
               cache_names=("annT", "amTs"))
    return outs[0]


# revision 11
# speedup vs baseline: 2560.4012x; 3.3355x over previous
"""2D DCT-II (4096x4096) on 8 Trainium2 NeuronCores (axon/PJRT SPMD).

Math: the reference computes C = A_M @ x @ A_N^T, where y = x[pm][:, pn]
(Makhoul even-odd reorder) is folded into the tables (A[:, pm[i]] = G[:, i]):
  G_M[u,i] = 0.5*(eMr[u]*cos(2pi*u*i/M) + eMi[u]*sin(2pi*u*i/M))
  G_N[v,j] = 2.0*(eNr[v]*cos(2pi*v*j/N) + eNi[v]*sin(2pi*v*j/N))
On device (per core k, rows_k = 512k..512k+512):
  AllGather x (each core gets the full 4096x4096 x in HBM), then locally
  T1^T = (A_M[rows_k,:] @ x)^T  via stationary=x-tiles, moving=amTs (SBUF),
  C[rows_k,:] = T1 @ A_N^T      via stationary=T1^T-tiles, moving=annT.
Input x is row-sharded (zero host copies), output C is row-sharded (the
fetched stacked array IS C). Tables are cached on device across calls.
Everything moves as bf16 (wire + HBM streams); matmuls accumulate in
fp32 PSUM. End-to-end rel err ~4e-3 vs the 2e-2 gate.
"""
import numpy as np

_NCORES = 8
_SZ = 4096
_RPC = _SZ // _NCORES  # 512 rows per core
_KT = _SZ // 128       # 32 contraction tiles

_state = {}


def _bf16():
    import ml_dtypes
    return ml_dtypes.bfloat16


# --------------------------------------------------------------------------
# Bass kernel
# --------------------------------------------------------------------------
def _build_bass():
    import concourse.bacc as bacc
    import concourse.mybir as mybir
    from concourse.tile import TileContext

    fp32 = mybir.dt.float32
    bf16 = mybir.dt.bfloat16
    VP = 512               # moving-panel width (max moving free dim)
    NVP = _SZ // VP        # 8
    UT = _RPC // 128       # 4 u-tiles per core
    CB = 4                 # concurrent PSUM accumulators in phase 1

    nc = bacc.Bacc("TRN2", target_bir_lowering=False, debug=False,
                   num_devices=_NCORES)
    xk = nc.declare_dram_parameter("xk", [_RPC, _SZ], bf16, isOutput=False)
    annT = nc.declare_dram_parameter("annT", [_SZ, _SZ], bf16, isOutput=False)
    amTs = nc.declare_dram_parameter("amTs", [_SZ, _RPC], bf16, isOutput=False)
    cout = nc.declare_dram_parameter("cout", [_RPC, _SZ], bf16, isOutput=True)

    xg_send = nc.dram_tensor("xg_send", [_RPC, _SZ], bf16)
    xg_full = nc.dram_tensor("xg_full", [_SZ, _SZ], bf16,
                             addr_space="Shared")

    with TileContext(nc) as tc:
        # bounce x_k into internal DRAM (collectives can't touch kernel I/O)
        nc.sync.dma_start(out=xg_send[:], in_=xk[:])

        nc.gpsimd.collective_compute(
            "AllGather",
            mybir.AluOpType.bypass,
            ins=[xg_send[:]],
            outs=[xg_full[:]],
            replica_groups=[list(range(_NCORES))],
        )

        with (
            tc.tile_pool(name="ams", bufs=1) as ams_pool,
            tc.tile_pool(name="t1", bufs=1) as t1_pool,
            tc.tile_pool(name="xp", bufs=4) as xp_pool,
            tc.tile_pool(name="anp", bufs=4) as anp_pool,
            tc.tile_pool(name="ps", bufs=8, space="PSUM") as ps_pool,
            tc.tile_pool(name="ev", bufs=4) as ev_pool,
        ):
            # amTs resident in SBUF: ams[i0, it, u] = amTs[it*128+i0, u]
            ams = ams_pool.tile([128, _KT * _RPC], bf16)  # 4 MB
            nc.sync.dma_start(
                out=ams[:].rearrange("p (it u) -> p it u", it=_KT),
                in_=amTs[:].rearrange("(it p) u -> p it u", p=128))
            # T1^T resident: t1[c0, ct, u] = T1[u, ct*128+c0]
            t1 = t1_pool.tile([128, _KT * _RPC], bf16)    # 4 MB

            # phase 1: T1^T[c, u] = sum_i x[i, c] * amTs[i, u]
            # x streamed in coalesced [128, 4 it-tiles, 512] chunks (512 KB)
            for cb in range(_SZ // (CB * 128)):           # 8 column blocks
                pss = [ps_pool.tile([128, _RPC], fp32, tag="ps",
                                    name=f"ps_{cb}_{ci}")
                       for ci in range(CB)]
                for it4 in range(_KT // 4):
                    xp = xp_pool.tile([128, 4 * CB * 128], bf16, tag="xp")
                    nc.sync.dma_start(
                        out=xp[:].rearrange("p (s c) -> p s c", s=4),
                        in_=xg_full[it4 * 512:(it4 + 1) * 512,
                                    cb * CB * 128:(cb + 1) * CB * 128]
                        .rearrange("(s p) c -> p s c", p=128))
                    for s in range(4):
                        it = it4 * 4 + s
                        for ci in range(CB):
                            nc.tensor.matmul(
                                pss[ci][:],
                                xp[:, s * CB * 128 + ci * 128:
                                      s * CB * 128 + (ci + 1) * 128],
                                ams[:, it * _RPC:(it + 1) * _RPC],
                                start=(it == 0), stop=(it == _KT - 1))
                for ci in range(CB):
                    ct = cb * CB + ci
                    nc.vector.tensor_copy(
                        t1[:, ct * _RPC:(ct + 1) * _RPC], pss[ci][:])

            # phase 2: C[u, v] = sum_c T1^T[c, u] * annT[c, v]
            for vp in range(NVP):                         # 8 v-panels
                qss = [ps_pool.tile([128, VP], fp32, tag="ps",
                                    name=f"qs_{vp}_{ut}")
                       for ut in range(UT)]
                for ct4 in range(_KT // 4):
                    anp = anp_pool.tile([128, 4 * VP], bf16, tag="anp")
                    nc.sync.dma_start(
                        out=anp[:].rearrange("p (s v) -> p s v", s=4),
                        in_=annT[ct4 * 512:(ct4 + 1) * 512,
                                 vp * VP:(vp + 1) * VP]
                        .rearrange("(s p) v -> p s v", p=128))
                    for s in range(4):
                        ct = ct4 * 4 + s
                        for ut in range(UT):
                            nc.tensor.matmul(
                                qss[ut][:],
                                t1[:, ct * _RPC + ut * 128:
                                      ct * _RPC + (ut + 1) * 128],
                                anp[:, s * VP:(s + 1) * VP],
                                start=(ct == 0), stop=(ct == _KT - 1))
                for ut in range(UT):
                    ev = ev_pool.tile([128, VP], bf16, tag="ev")
                    nc.vector.tensor_copy(ev[:], qss[ut][:])
                    nc.sync.dma_start(
                        out=cout[ut * 128:(ut + 1) * 128,
                                 vp * VP:(vp + 1) * VP],
                        in_=ev[:])

    nc.compile()
    return nc


# --------------------------------------------------------------------------
# PJRT SPMD runner (compile once, run many)
# --------------------------------------------------------------------------
def _build_runner(nc, n_cores, replicated_names=()):
    import jax
    from jax.sharding import Mesh, NamedSharding, PartitionSpec
    from jax.experimental.shard_map import shard_map
    import concourse.mybir as mybir
    from concourse import bass2jax
    from concourse.bass2jax import _bass_exec_p, partition_id_tensor

    bass2jax.install_neuronx_cc_hook()
    partition_name = (nc.partition_id_tensor.name
                      if nc.partition_id_tensor else None)

    in_names, out_names, out_avals, zero_outs = [], [], [], []
    for alloc in nc.m.functions[0].allocations:
        if not isinstance(alloc, mybir.MemoryLocationSet):
            continue
        name = alloc.memorylocations[0].name
        if alloc.kind == "ExternalInput":
            if name != partition_name:
                in_names.append(name)
        elif alloc.kind == "ExternalOutput":
            shape = tuple(alloc.tensor_shape)
            dtype = mybir.dt.np(alloc.dtype)
            out_names.append(name)
            out_avals.append(jax.core.ShapedArray(shape, dtype))
            zero_outs.append(np.zeros(shape, dtype))
    n_params = len(in_names)
    n_outs = len(out_avals)
    in_names_all = list(in_names) + out_names
    if partition_name is not None:
        in_names_all = in_names_all + [partition_name]
    donate = tuple(range(n_params, n_params + n_outs))

    def _body(*args):
        operands = list(args)
        if partition_name is not None:
            operands.append(partition_id_tensor())
        outs = _bass_exec_p.bind(
            *operands,
            out_avals=tuple(out_avals),
            in_names=tuple(in_names_all),
            out_names=tuple(out_names),
            lowering_input_output_aliases=(),
            sim_require_finite=True,
            sim_require_nnan=True,
            nc=nc,
        )
        return tuple(outs)

    devices = jax.devices()[:n_cores]
    mesh = Mesh(np.asarray(devices), ("core",))
    spec_row = PartitionSpec("core")
    spec_rep = PartitionSpec()
    in_specs = tuple(spec_rep if nm in replicated_names else spec_row
                     for nm in in_names)
    sharded = jax.jit(
        shard_map(_body, mesh=mesh,
                  in_specs=in_specs + (spec_row,) * n_outs,
                  out_specs=(spec_row,) * n_outs,
                  check_rep=False),
        donate_argnums=donate, keep_unused=True)

    shard = NamedSharding(mesh, spec_row)
    shard_rep = NamedSharding(mesh, spec_rep)
    _dev_cache = {}

    import jax.numpy as jnp
    _zero_shapes = [(n_cores * z.shape[0], *z.shape[1:]) for z in zero_outs]
    _zero_dtypes = [z.dtype for z in zero_outs]

    _make_zeros = jax.jit(
        lambda: tuple(jnp.zeros(s, d)
                      for s, d in zip(_zero_shapes, _zero_dtypes)),
        out_shardings=(shard,) * len(_zero_shapes))

    _prev_outs = [None]

    def run(stacked_in, cache_names=(), fetch=True, block=True):
        """stacked_in: dict name -> FULL stacked np array (replicated names
        get the per-core array as-is)."""
        concat_in = []
        for name in in_names:
            if name in cache_names and name in _dev_cache:
                concat_in.append(_dev_cache[name])
                continue
            sh = shard_rep if name in replicated_names else shard
            arr = jax.device_put(stacked_in[name], sh)
            if name in cache_names:
                jax.block_until_ready(arr)
                _dev_cache[name] = arr
            concat_in.append(arr)
        # donate the previous call's output buffers back as this call's
        # (write-only) output operands; first call uses fresh zeros
        outs_in = _prev_outs[0]
        if outs_in is None:
            outs_in = _make_zeros()
        raw = sharded(*concat_in, *outs_in)
        _prev_outs[0] = raw
        if not fetch:
            if block:
                jax.block_until_ready(raw)
            return raw
        return [np.asarray(o) for o in raw]

    run.dev_cache = _dev_cache
    run.out_names = out_names
    return run


# --------------------------------------------------------------------------
# host-side tables
# --------------------------------------------------------------------------
def _tables(expkM, expkN):
    key = (expkM.tobytes(), expkN.tobytes())
    cached = _state.get("tables")
    if cached is not None and cached[0] == key:
        return cached[1], cached[2]
    run = _state.get("run")
    if run is not None:
        run.dev_cache.clear()
    bf16 = _bf16()
    n = _SZ
    i = np.arange(n)
    pm = np.where(i < (n + 1) // 2, 2 * i, 2 * (n - i) - 1)
    pinv = np.empty(n, dtype=np.int64)
    pinv[pm] = i
    # Cp[j, v] = cos(2pi * pinv[j] * v / n)
    ang = (2.0 * np.pi / n) * np.outer(pinv.astype(np.float64),
                                       i.astype(np.float64))
    Cp = np.cos(ang)
    Sp = np.sin(ang)
    eMr = expkM[:, 0].astype(np.float64)
    eMi = expkM[:, 1].astype(np.float64)
    eNr = expkN[:, 0].astype(np.float64)
    eNi = expkN[:, 1].astype(np.float64)
    annT = (2.0 * (Cp * eNr[None, :] + Sp * eNi[None, :])).astype(bf16)
    amT = (0.5 * (Cp * eMr[None, :] + Sp * eMi[None, :])).astype(bf16)
    # amTs stacked: core k gets amT[:, k*512:(k+1)*512]
    amTs = np.ascontiguousarray(
        amT.reshape(n, _NCORES, _RPC).transpose(1, 0, 2)
    ).reshape(_NCORES * n, _RPC)
    _state["tables"] = (key, annT, amTs)
    return annT, amTs


def kernel(x, expkM, expkN, M, N):
    bf16 = _bf16()
    x = np.asarray(x, dtype=np.float32).astype(bf16)
    expkM = np.asarray(expkM, dtype=np.float32)
    expkN = np.asarray(expkN, dtype=np.float32)
    assert x.shape == (_SZ, _SZ)

    annT, amTs = _tables(expkM, expkN)
    if "run" not in _state:
        _state["run"] = _build_runner(_build_bass(), _NCORES,
                                      replicated_names=("annT",))
    run = _state["run"]

    outs = run({"xk": x, "annT": annT, "amTs": amTs},
               cache_names=("annT", "amTs"))
    return outs[0].astype(np.float32)
